# revision 27
# baseline (speedup 1.0000x reference)
"""Trainium2 Bass kernel for the BasicRNN problem.

Computation (see harness reference):
    E  = x @ Wp.T + bp                       # (B, S) sensory drive
    Z0 = 0                                   # (B, TOTAL)
    for t in range(time_steps):
        inj = [E if t % 5 == 0 else 0 | 0 | 0]
        Z  = relu(Z @ W + inj)
    out = Z[:, S+I:] @ Wo.T + bo             # (B, num_classes)

Strategy: data-parallel over batch across 8 NeuronCores (256 rows/core).
Each core keeps the state TRANSPOSED in SBUF (ZT: [TOTAL, B_shard] as
32 [128, 256] tiles) so every step is
    ZT_next[m] = relu(sum_k W[k, m-block].T-free @ ZT[k] (+ E^T[m]))
with lhsT = a [128k, 128m] block of W in its natural layout — no
transposes anywhere.  The kernel is jointly PE- and DMA-limited when W
(32 MiB fp16) is streamed every step, so the last 12 of the 32
column panels (12 MiB) stay RESIDENT in SBUF after step 2 — later
steps stream only 20 panels and the final step (O block only) streams
none.  The step-1 row-prefix of W is host-packed into one contiguous
block that pre-loads into the resident region, so step 1 runs at the
PE floor with the big stream pool free to prefetch step-2 panels.
Head rules learned from traces: full-128-partition DMAs only
(partition-sliced ones crawl), the tiny bias DMA must be issued before
the 8 MiB prefix block (ring semaphores serialize issue order), the
ACT table and HAM clock ramp are pre-warmed, and the output-layer
matmuls in the last step lag one m-tile so they never wait on a relu.
Every step then runs at the 109.2 ns/matmul hardware floor
(N/2.4 GHz + NX issue) with zero stalls.  Max rel err ~8e-4 vs the
fp32 reference.  fp8/int8 were evaluated and rejected: e4m3 W+state
gives 7.5e-2 final error (gate is 2e-2); DoubleRow also only pays at
free-dim >= 512 which this batch shard cannot reach.
"""

import numpy as np
from contextlib import ExitStack

from concourse import bacc, tile, mybir
from concourse.bass_utils import run_bass_kernel_spmd

P = 128
N_CORES = 8
F32 = mybir.dt.float32
F16 = mybir.dt.float16
AF = mybir.ActivationFunctionType

_cache: dict = {}

# extra kwargs for run_bass_kernel_spmd (test harness sets e.g. trace=True)
RUN_KWARGS: dict = {}
LAST_RESULT = None

N_RES = 12  # W panels kept resident in SBUF (of MT total)


def _emit(ctx: ExitStack, tc, aps, cfg):
    (B, IN_DIM, S, I_DIM, O_DIM, NCP, steps) = cfg
    TOT = S + I_DIM + O_DIM
    KT = TOT // P           # contraction tiles per step
    MT = TOT // P           # output-block tiles per step
    ST = S // P             # sensory tiles
    OT = O_DIM // P         # O-block tiles
    O0 = (S + I_DIM) // P   # first m-tile of the O block
    INT = IN_DIM // P
    nc = tc.nc
    (xwp, bp, Wpre, Wpk, WoT, out) = aps

    n_res = min(N_RES, MT)
    RES0 = MT - n_res       # panels m >= RES0 are resident

    io_pool = ctx.enter_context(tc.tile_pool(name="io", bufs=1))
    state_pool = ctx.enter_context(tc.tile_pool(name="state", bufs=1))
    e_pool = ctx.enter_context(tc.tile_pool(name="e", bufs=1))
    w_pool = ctx.enter_context(tc.tile_pool(name="w", bufs=5))
    res_pool = ctx.enter_context(tc.tile_pool(name="res", bufs=1))
    ps_pool = ctx.enter_context(tc.tile_pool(name="ps", bufs=4, space="PSUM"))

    def psum_tile(n):
        # all PSUM tiles share one tag (one bank each, `bufs` slots)
        return ps_pool.tile([P, 512], F32, name="ps", tag="ps")[:, :n]
    o_pool = ctx.enter_context(tc.tile_pool(name="o", bufs=2))

    # one contiguous region for the resident panels; its first columns
    # double as the landing zone for the host-packed step-1 prefix block
    res_big = res_pool.tile([P, n_res * TOT], F16, name="res")
    res_sb = [res_big[:, j * TOT:(j + 1) * TOT] for j in range(n_res)]
    pre_cols = MT * ST * P
    use_pre = steps >= 2 and n_res * TOT >= pre_cols

    # ---- PE warm-up: dummy matmuls with no DMA deps run during the
    # initial input DMAs and trip the HAM clock un-throttle (~3.4 us of
    # sustained PE activity) so the real matmuls start at 2.4 GHz.
    # DVE memset (gpsimd takes ~7 us to start and stalls the warm-up).
    # NOTE: wu must stay [P, P] — growing it shifts every later SBUF
    # allocation and lands the kernel in an SBUF layout whose bank
    # conflicts cost +22 ns on EVERY matmul (109 -> 131 ns spacing).
    wu = io_pool.tile([P, P], F16, name="wu")
    nc.vector.memset(wu[:], 0.0)
    # warm the ACT table too: the first ACTIVATE otherwise pays a
    # 1.3 us ACT_TABLE_LOAD on the critical path of the t=0 relu
    nc.scalar.activation(wu[:, :1], wu[:, :1], AF.Relu)
    for _ in range(32):
        wu_ps = psum_tile(P)
        nc.tensor.matmul(wu_ps[:], lhsT=wu[:], rhs=wu[:], start=True, stop=True)

    # ---- load the small operands in as few DMAs as possible.  x/Wp are
    # host-packed into one [P, INT*(B+S)] buffer so the load is two DMAs
    # with 20 KB-contiguous partition rows (full HBM bandwidth); its SBUF
    # space is later recycled for the step-1 W prefixes and the Wo tiles.
    wot_sb = []
    BS = B + S
    # bp FIRST: DMA issues serialize on ~9 rotating ring semaphores, so
    # this tiny transfer must not queue behind the 8 MiB prefix block
    # (the t=0 relu needs it as its bias)
    bp_t = io_pool.tile([P, ST], F32, name="bp_t")
    nc.sync.dma_start(bp_t[:], bp)
    bp_sb = [bp_t[:, i:i + 1] for i in range(ST)]
    xwp_t = io_pool.tile([P, INT * BS], F16, name="xwp_t")
    xt_sb = [xwp_t[:, i * BS:i * BS + B] for i in range(INT)]
    wpt_sb = [xwp_t[:, i * BS + B:(i + 1) * BS] for i in range(INT)]
    # per-k-chunk DMAs: full-128-partition transfers are the fast DMA
    # path (partition-sliced ones crawl), and chunking lets the E
    # projection start on chunk 0 while the rest stream in.  Chunk 0 is
    # split 4 ways (4 rings drain ~4x faster) since its arrival gates
    # the first E matmul whenever the framework preamble runs short.
    q = BS // 4
    for si in range(4):
        lo = si * q
        hi = BS if si == 3 else (si + 1) * q
        nc.sync.dma_start(xwp_t[:, lo:hi], xwp[:, lo:hi])
    for i in range(1, INT):
        nc.sync.dma_start(xwp_t[:, i * BS:(i + 1) * BS],
                          xwp[:, i * BS:(i + 1) * BS])
    # step-1 prefix block (rows 0:S of every panel, host-packed
    # m-contiguous): lands in the resident region before step 1 needs it,
    # in column chunks so panels complete in consumption order
    if use_pre:
        CPC = 2 * ST * P           # 2 panels' prefixes per column chunk
        for c0 in range(0, pre_cols, CPC):
            cw = min(CPC, pre_cols - c0)
            nc.sync.dma_start(res_big[:, c0:c0 + cw], Wpre[:, c0:c0 + cw])
    n_slots = (INT * BS) // (ST * P)
    sm_used = [0]

    def sm_tile(width):
        # recycle a slot of the x/Wp buffer (rotating; Tile's subslice
        # dependency tracking serializes against any overlapping readers)
        s = sm_used[0] % n_slots
        sm_used[0] += 1
        return xwp_t[:, s * (ST * P): s * (ST * P) + width]

    # ---- E^T = Wp @ x_shard^T + bp  (fp32, [S, B] as ST tiles)
    # k-outer: each input k-tile is consumed as soon as its DMA lands,
    # so the E matmuls overlap their own input loads
    e_ps = [ps_pool.tile([P, 512], F32, name="pse", tag="ps" if m < ST // 2
            else "ps2")[:, :B] for m in range(ST)]
    for ki in range(INT):
        for m in range(ST):
            nc.tensor.matmul(
                e_ps[m][:],
                lhsT=wpt_sb[ki][:, m * P:(m + 1) * P],
                rhs=xt_sb[ki][:],
                start=(ki == 0),
                stop=(ki == INT - 1),
            )
    # ---- state: two ping-pong buffers of KT [128, B] fp16 tiles
    zt = [
        [state_pool.tile([P, B], F16, name=f"z{b}_{k}") for k in range(KT)]
        for b in (0, 1)
    ]

    # t = 0: Z1 = relu([E | 0 | 0]) straight from PSUM with the bias fused
    # into the activation (ACT), while DVE independently materializes the
    # un-relu'd E for the t=5 injection — two short parallel chains
    # instead of one 16-op serial ACT chain.
    e_sb = []
    for m in range(ST):
        if steps >= 1:
            nc.scalar.activation(zt[1][m][:], e_ps[m][:], AF.Relu,
                                 bias=bp_sb[m][:])
        et = e_pool.tile([P, B], F32, name=f"et{m}")
        nc.vector.tensor_scalar_add(et[:], e_ps[m][:], bp_sb[m][:])
        e_sb.append(et)

    fin = steps % 2
    chunks = [(bt, c0) for bt in range(B // P) for c0 in range(0, NCP, 512)]
    ps2_tiles = {}

    def load_wot():
        for i in range(OT):
            if NCP <= ST * P and n_slots >= 1:
                t4 = sm_tile(NCP)
            else:
                t4 = io_pool.tile([P, NCP], F16, name=f"wot{i}")
            nc.sync.dma_start(t4[:, :NCP], WoT[i])
            wot_sb.append(t4)

    def emit_final_group(j, last):
        # one k-slice (state tile O0+j) of the output-layer matmuls; called
        # inside the last step so these fold into the recurrence pipeline
        for (bt, c0) in chunks:
            cw = min(512, NCP - c0)
            if j == 0:
                ps2_tiles[(bt, c0)] = ps_pool.tile(
                    [P, 512], F32, name="ps2", tag="ps2")[:, :cw]
            nc.tensor.matmul(
                ps2_tiles[(bt, c0)][:],
                lhsT=zt[fin][O0 + j][:, bt * P:(bt + 1) * P],
                rhs=wot_sb[j][:, c0:c0 + cw],
                start=(j == 0),
                stop=last,
            )
            if last:
                ot = o_pool.tile([P, 512], F32, name="obuf")
                if (bt + c0 // 512) % 2 == 0:
                    nc.vector.tensor_copy(ot[:, :cw], ps2_tiles[(bt, c0)][:])
                else:
                    nc.scalar.activation(ot[:, :cw], ps2_tiles[(bt, c0)][:],
                                         AF.Copy)
                nc.sync.dma_start(out[bt, :, c0:c0 + cw], ot[:, :cw])

    live = ST  # number of non-zero k tiles in the current state
    for t in range(1, steps):
        cur, nxt = t % 2, (t + 1) % 2
        is_last = t == steps - 1
        m_lo, m_hi = (O0, MT) if is_last else (0, MT)
        k_n = live
        inject = (t % 5 == 0)
        if t == steps - 2 or (is_last and not wot_sb):
            # output-layer weights: issue the DMAs a step early so they
            # sit ready when the last step starts draining
            load_wot()
        for m in range(m_lo, m_hi):
            if m >= RES0 and t >= 2:
                # resident panel: loaded once at t=2, reused afterwards
                wp = res_sb[m - RES0]
                if t == 2:
                    nc.sync.dma_start(wp[:, : k_n * P], Wpk[m, :, : k_n * P])
            elif t == 1 and use_pre:
                # step-1 prefix: already pre-loaded in the resident
                # region by the head DMA block (overwritten at t=2)
                wp = res_big[:, m * (ST * P):(m + 1) * (ST * P)]
            else:
                wp = w_pool.tile([P, TOT], F16, name="wp")
                nc.sync.dma_start(wp[:, : k_n * P], Wpk[m, :, : k_n * P])
            ps = psum_tile(B)
            for k in range(k_n):
                nc.tensor.matmul(
                    ps[:],
                    lhsT=wp[:, k * P:(k + 1) * P],
                    rhs=zt[cur][k][:],
                    start=(k == 0),
                    stop=(k == k_n - 1),
                )
            if inject and m < ST:
                nc.vector.tensor_add(ps[:], ps[:], e_sb[m][:])
            if m % 2 == 0:
                nc.scalar.activation(zt[nxt][m][:], ps[:], AF.Relu)
            else:
                nc.vector.tensor_scalar_max(zt[nxt][m][:], ps[:], 0.0)
            if is_last and m > m_lo:
                # lag the output-layer slice one m-tile so it never waits
                # on the relu that produced its lhsT
                emit_final_group(m - 1 - O0, last=False)
        if is_last:
            emit_final_group(m_hi - 1 - O0, last=True)
        live = KT

    if steps < 2:
        # the O block was never written; zero it and run the output layer
        for j in range(OT):
            nc.vector.memset(zt[fin][O0 + j][:], 0.0)
        if not wot_sb:
            load_wot()
        for j in range(OT):
            emit_final_group(j, last=(j == OT - 1))


def _build(cfg):
    (B, IN_DIM, S, I_DIM, O_DIM, NCP, steps) = cfg
    TOT = S + I_DIM + O_DIM
    nc = bacc.Bacc("TRN2", target_bir_lowering=False, debug=False,
                   num_devices=N_CORES)
    xwp = nc.dram_tensor("xwp", (P, (IN_DIM // P) * (B + S)), F16, kind="ExternalInput").ap()
    bp = nc.dram_tensor("bp", (P, S // P), F32, kind="ExternalInput").ap()
    Wpre = nc.dram_tensor("Wpre", (P, (TOT // P) * S), F16, kind="ExternalInput").ap()
    Wpk = nc.dram_tensor("Wpk", (TOT // P, P, TOT), F16, kind="ExternalInput").ap()
    WoT = nc.dram_tensor("WoT", (O_DIM // P, P, NCP), F16, kind="ExternalInput").ap()
    out = nc.dram_tensor("out", (B // P, P, NCP), F32, kind="ExternalOutput").ap()
    with ExitStack() as ctx, tile.TileContext(nc) as tc:
        with ExitStack() as inner:
            _emit(inner, tc, (xwp, bp, Wpre, Wpk, WoT, out), cfg)
    nc.compile()
    return nc


def _get_nc(cfg):
    if cfg not in _cache:
        _cache[cfg] = _build(cfg)
    return _cache[cfg]


def kernel(x, W, Wp, bp, Wo, bo, time_steps):
    x = np.asarray(x, dtype=np.float32)
    W = np.asarray(W, dtype=np.float32)
    Wp = np.asarray(Wp, dtype=np.float32)
    bp = np.asarray(bp, dtype=np.float32)
    Wo = np.asarray(Wo, dtype=np.float32)
    bo = np.asarray(bo, dtype=np.float32)
    steps = int(time_steps)

    B_full, IN_DIM = x.shape
    TOT = W.shape[0]
    S = Wp.shape[0]
    NCLS, O_DIM = Wo.shape
    assert B_full % (N_CORES * P) == 0 and IN_DIM % P == 0
    assert S % P == 0 and O_DIM % P == 0 and TOT % P == 0
    B = B_full // N_CORES
    NCP = ((NCLS + P - 1) // P) * P
    cfg = (B, IN_DIM, S, TOT - S - O_DIM, O_DIM, NCP, steps)

    nc = _get_nc(cfg)

    # ---- host packing (replicated operands)
    W16 = W.astype(np.float16)
    # Wpk[mt, p, kt*P + mf] = W[kt*P + p, mt*P + mf]
    Wpk = np.ascontiguousarray(
        W16.reshape(TOT // P, P, TOT // P, P).transpose(2, 1, 0, 3)
    ).reshape(TOT // P, P, TOT)
    # step-1 prefix block: rows 0:S of every panel, panel-contiguous
    Wpre = np.ascontiguousarray(
        Wpk[:, :, :S].transpose(1, 0, 2)).reshape(P, (TOT // P) * S)
    WpT16 = Wp.T.astype(np.float16).reshape(IN_DIM // P, P, S)
    bpt = np.ascontiguousarray(bp.reshape(S // P, P).T)  # [p, s_tile]
    WoTp = np.zeros((O_DIM, NCP), dtype=np.float16)
    WoTp[:, :NCLS] = Wo.T.astype(np.float16)
    WoT = WoTp.reshape(O_DIM // P, P, NCP)

    xT = x.T.astype(np.float16)  # (IN_DIM, B_full)
    in_maps = []
    for c in range(N_CORES):
        xc = xT[:, c * B:(c + 1) * B].reshape(IN_DIM // P, P, B)
        xwp = np.concatenate([xc, WpT16], axis=2)  # (INT, P, B+S)
        # big-row layout: [P, INT*(B+S)] so the load is 2 full-BW DMAs
        xwp = np.ascontiguousarray(xwp.transpose(1, 0, 2)).reshape(P, -1)
        in_maps.append({"xwp": xwp, "bp": bpt, "Wpre": Wpre, "Wpk": Wpk,
                        "WoT": WoT})

    global LAST_RESULT
    res = run_bass_kernel_spmd(nc, in_maps, core_ids=list(range(N_CORES)),
                               **RUN_KWARGS)
    LAST_RESULT = res

    outs = []
    for c in range(N_CORES):
        oc = res.results[c]["out"].reshape(B, NCP)[:, :NCLS]
        outs.append(oc)
    return (np.concatenate(outs, axis=0) + bo[None, :]).astype(np.float32)


# revision 28
# speedup vs baseline: 1.0016x; 1.0016x over previous
"""Trainium2 Bass kernel for the BasicRNN problem.

Computation (see harness reference):
    E  = x @ Wp.T + bp                       # (B, S) sensory drive
    Z0 = 0                                   # (B, TOTAL)
    for t in range(time_steps):
        inj = [E if t % 5 == 0 else 0 | 0 | 0]
        Z  = relu(Z @ W + inj)
    out = Z[:, S+I:] @ Wo.T + bo             # (B, num_classes)

Strategy: data-parallel over batch across 8 NeuronCores (256 rows/core).
Each core keeps the state TRANSPOSED in SBUF (ZT: [TOTAL, B_shard] as
32 [128, 256] tiles) so every step is
    ZT_next[m] = relu(sum_k W[k, m-block].T-free @ ZT[k] (+ E^T[m]))
with lhsT = a [128k, 128m] block of W in its natural layout — no
transposes anywhere.  The kernel is jointly PE- and DMA-limited when W
(32 MiB fp16) is streamed every step, so the last 12 of the 32
column panels (12 MiB) stay RESIDENT in SBUF after step 2 — later
steps stream only 20 panels and the final step (O block only) streams
none.  The step-1 row-prefix of W is host-packed into one contiguous
block that pre-loads into the resident region, so step 1 runs at the
PE floor with the big stream pool free to prefetch step-2 panels.
Head rules learned from traces: full-128-partition DMAs only
(partition-sliced ones crawl), the tiny bias DMA must be issued before
the 8 MiB prefix block (ring semaphores serialize issue order), the
ACT table and HAM clock ramp are pre-warmed, and the output-layer
matmuls in the last step lag one m-tile so they never wait on a relu.
Every step then runs at the 109.2 ns/matmul hardware floor
(N/2.4 GHz + NX issue) with zero stalls.  Max rel err ~8e-4 vs the
fp32 reference.  fp8/int8 were evaluated and rejected: e4m3 W+state
gives 7.5e-2 final error (gate is 2e-2); DoubleRow also only pays at
free-dim >= 512 which this batch shard cannot reach.
"""

import numpy as np
from contextlib import ExitStack

from concourse import bacc, tile, mybir
from concourse.bass_utils import run_bass_kernel_spmd

P = 128
N_CORES = 8
F32 = mybir.dt.float32
F16 = mybir.dt.float16
AF = mybir.ActivationFunctionType

_cache: dict = {}

# extra kwargs for run_bass_kernel_spmd (test harness sets e.g. trace=True)
RUN_KWARGS: dict = {}
LAST_RESULT = None

N_RES = 12  # W panels kept resident in SBUF (of MT total)


def _emit(ctx: ExitStack, tc, aps, cfg):
    (B, IN_DIM, S, I_DIM, O_DIM, NCP, steps) = cfg
    TOT = S + I_DIM + O_DIM
    KT = TOT // P           # contraction tiles per step
    MT = TOT // P           # output-block tiles per step
    ST = S // P             # sensory tiles
    OT = O_DIM // P         # O-block tiles
    O0 = (S + I_DIM) // P   # first m-tile of the O block
    INT = IN_DIM // P
    nc = tc.nc
    (xwp, bp, Wpre, Wpk, WoT, out) = aps

    n_res = min(N_RES, MT)
    RES0 = MT - n_res       # panels m >= RES0 are resident

    io_pool = ctx.enter_context(tc.tile_pool(name="io", bufs=1))
    state_pool = ctx.enter_context(tc.tile_pool(name="state", bufs=1))
    e_pool = ctx.enter_context(tc.tile_pool(name="e", bufs=1))
    w_pool = ctx.enter_context(tc.tile_pool(name="w", bufs=5))
    res_pool = ctx.enter_context(tc.tile_pool(name="res", bufs=1))
    ps_pool = ctx.enter_context(tc.tile_pool(name="ps", bufs=4, space="PSUM"))

    def psum_tile(n):
        # all PSUM tiles share one tag (one bank each, `bufs` slots)
        return ps_pool.tile([P, 512], F32, name="ps", tag="ps")[:, :n]
    o_pool = ctx.enter_context(tc.tile_pool(name="o", bufs=2))

    # one contiguous region for the resident panels; its first columns
    # double as the landing zone for the host-packed step-1 prefix block
    res_big = res_pool.tile([P, n_res * TOT], F16, name="res")
    res_sb = [res_big[:, j * TOT:(j + 1) * TOT] for j in range(n_res)]
    pre_cols = MT * ST * P
    use_pre = steps >= 2 and n_res * TOT >= pre_cols

    # ---- PE warm-up: dummy matmuls with no DMA deps run during the
    # initial input DMAs and trip the HAM clock un-throttle (~3.4 us of
    # sustained PE activity) so the real matmuls start at 2.4 GHz.
    # DVE memset (gpsimd takes ~7 us to start and stalls the warm-up).
    # NOTE: wu must stay [P, P] — growing it shifts every later SBUF
    # allocation and lands the kernel in an SBUF layout whose bank
    # conflicts cost +22 ns on EVERY matmul (109 -> 131 ns spacing).
    wu = io_pool.tile([P, P], F16, name="wu")
    nc.vector.memset(wu[:], 0.0)
    # warm the ACT table too: the first ACTIVATE otherwise pays a
    # 1.3 us ACT_TABLE_LOAD on the critical path of the t=0 relu
    nc.scalar.activation(wu[:, :1], wu[:, :1], AF.Relu)
    for _ in range(32):
        wu_ps = psum_tile(P)
        nc.tensor.matmul(wu_ps[:], lhsT=wu[:], rhs=wu[:], start=True, stop=True)

    # ---- load the small operands in as few DMAs as possible.  x/Wp are
    # host-packed into one [P, INT*(B+S)] buffer so the load is two DMAs
    # with 20 KB-contiguous partition rows (full HBM bandwidth); its SBUF
    # space is later recycled for the step-1 W prefixes and the Wo tiles.
    wot_sb = []
    BS = B + S
    # bp FIRST: DMA issues serialize on ~9 rotating ring semaphores, so
    # this tiny transfer must not queue behind the 8 MiB prefix block
    # (the t=0 relu needs it as its bias)
    bp_t = io_pool.tile([P, ST], F32, name="bp_t")
    nc.sync.dma_start(bp_t[:], bp)
    bp_sb = [bp_t[:, i:i + 1] for i in range(ST)]
    xwp_t = io_pool.tile([P, INT * BS], F16, name="xwp_t")
    xt_sb = [xwp_t[:, i * BS:i * BS + B] for i in range(INT)]
    wpt_sb = [xwp_t[:, i * BS + B:(i + 1) * BS] for i in range(INT)]
    # per-k-chunk DMAs: full-128-partition transfers are the fast DMA
    # path (partition-sliced ones crawl), and chunking lets the E
    # projection start on chunk 0 while the rest stream in.  (Splitting
    # chunk 0 further was tried and is a net loss: the extra ~585 ns
    # Sync-queue issue slots delay every later DMA.)
    for i in range(INT):
        nc.sync.dma_start(xwp_t[:, i * BS:(i + 1) * BS],
                          xwp[:, i * BS:(i + 1) * BS])
    # step-1 prefix block (rows 0:S of every panel, host-packed
    # m-contiguous): lands in the resident region before step 1 needs it,
    # in column chunks so panels complete in consumption order
    if use_pre:
        CPC = 2 * ST * P           # 2 panels' prefixes per column chunk
        for c0 in range(0, pre_cols, CPC):
            cw = min(CPC, pre_cols - c0)
            nc.sync.dma_start(res_big[:, c0:c0 + cw], Wpre[:, c0:c0 + cw])
    n_slots = (INT * BS) // (ST * P)
    sm_used = [0]

    def sm_tile(width):
        # recycle a slot of the x/Wp buffer (rotating; Tile's subslice
        # dependency tracking serializes against any overlapping readers)
        s = sm_used[0] % n_slots
        sm_used[0] += 1
        return xwp_t[:, s * (ST * P): s * (ST * P) + width]

    # ---- E^T = Wp @ x_shard^T + bp  (fp32, [S, B] as ST tiles)
    # k-outer: each input k-tile is consumed as soon as its DMA lands,
    # so the E matmuls overlap their own input loads
    e_ps = [ps_pool.tile([P, 512], F32, name="pse", tag="ps" if m < ST // 2
            else "ps2")[:, :B] for m in range(ST)]
    for ki in range(INT):
        for m in range(ST):
            nc.tensor.matmul(
                e_ps[m][:],
                lhsT=wpt_sb[ki][:, m * P:(m + 1) * P],
                rhs=xt_sb[ki][:],
                start=(ki == 0),
                stop=(ki == INT - 1),
            )
    # ---- state: two ping-pong buffers of KT [128, B] fp16 tiles
    zt = [
        [state_pool.tile([P, B], F16, name=f"z{b}_{k}") for k in range(KT)]
        for b in (0, 1)
    ]

    # t = 0: Z1 = relu([E | 0 | 0]) straight from PSUM with the bias fused
    # into the activation (ACT), while DVE independently materializes the
    # un-relu'd E for the t=5 injection — two short parallel chains
    # instead of one 16-op serial ACT chain.
    e_sb = []
    for m in range(ST):
        if steps >= 1:
            nc.scalar.activation(zt[1][m][:], e_ps[m][:], AF.Relu,
                                 bias=bp_sb[m][:])
        et = e_pool.tile([P, B], F32, name=f"et{m}")
        nc.vector.tensor_scalar_add(et[:], e_ps[m][:], bp_sb[m][:])
        e_sb.append(et)

    fin = steps % 2
    chunks = [(bt, c0) for bt in range(B // P) for c0 in range(0, NCP, 512)]
    ps2_tiles = {}

    def load_wot():
        for i in range(OT):
            if NCP <= ST * P and n_slots >= 1:
                t4 = sm_tile(NCP)
            else:
                t4 = io_pool.tile([P, NCP], F16, name=f"wot{i}")
            nc.sync.dma_start(t4[:, :NCP], WoT[i])
            wot_sb.append(t4)

    def emit_final_group(j, last):
        # one k-slice (state tile O0+j) of the output-layer matmuls; called
        # inside the last step so these fold into the recurrence pipeline
        for (bt, c0) in chunks:
            cw = min(512, NCP - c0)
            if j == 0:
                ps2_tiles[(bt, c0)] = ps_pool.tile(
                    [P, 512], F32, name="ps2", tag="ps2")[:, :cw]
            nc.tensor.matmul(
                ps2_tiles[(bt, c0)][:],
                lhsT=zt[fin][O0 + j][:, bt * P:(bt + 1) * P],
                rhs=wot_sb[j][:, c0:c0 + cw],
                start=(j == 0),
                stop=last,
            )
            if last:
                ot = o_pool.tile([P, 512], F32, name="obuf")
                if (bt + c0 // 512) % 2 == 0:
                    nc.vector.tensor_copy(ot[:, :cw], ps2_tiles[(bt, c0)][:])
                else:
                    nc.scalar.activation(ot[:, :cw], ps2_tiles[(bt, c0)][:],
                                         AF.Copy)
                nc.sync.dma_start(out[bt, :, c0:c0 + cw], ot[:, :cw])

    live = ST  # number of non-zero k tiles in the current state
    for t in range(1, steps):
        cur, nxt = t % 2, (t + 1) % 2
        is_last = t == steps - 1
        m_lo, m_hi = (O0, MT) if is_last else (0, MT)
        k_n = live
        inject = (t % 5 == 0)
        if t == steps - 2 or (is_last and not wot_sb):
            # output-layer weights: issue the DMAs a step early so they
            # sit ready when the last step starts draining
            load_wot()
        for m in range(m_lo, m_hi):
            if m >= RES0 and t >= 2:
                # resident panel: loaded once at t=2, reused afterwards
                wp = res_sb[m - RES0]
                if t == 2:
                    nc.sync.dma_start(wp[:, : k_n * P], Wpk[m, :, : k_n * P])
            elif t == 1 and use_pre:
                # step-1 prefix: already pre-loaded in the resident
                # region by the head DMA block (overwritten at t=2)
                wp = res_big[:, m * (ST * P):(m + 1) * (ST * P)]
            else:
                wp = w_pool.tile([P, TOT], F16, name="wp")
                nc.sync.dma_start(wp[:, : k_n * P], Wpk[m, :, : k_n * P])
            ps = psum_tile(B)
            for k in range(k_n):
                nc.tensor.matmul(
                    ps[:],
                    lhsT=wp[:, k * P:(k + 1) * P],
                    rhs=zt[cur][k][:],
                    start=(k == 0),
                    stop=(k == k_n - 1),
                )
            if inject and m < ST:
                nc.vector.tensor_add(ps[:], ps[:], e_sb[m][:])
            if m % 2 == 0:
                nc.scalar.activation(zt[nxt][m][:], ps[:], AF.Relu)
            else:
                nc.vector.tensor_scalar_max(zt[nxt][m][:], ps[:], 0.0)
            if is_last and m > m_lo:
                # lag the output-layer slice one m-tile so it never waits
                # on the relu that produced its lhsT
                emit_final_group(m - 1 - O0, last=False)
        if is_last:
            emit_final_group(m_hi - 1 - O0, last=True)
        live = KT

    if steps < 2:
        # the O block was never written; zero it and run the output layer
        for j in range(OT):
            nc.vector.memset(zt[fin][O0 + j][:], 0.0)
        if not wot_sb:
            load_wot()
        for j in range(OT):
            emit_final_group(j, last=(j == OT - 1))


def _build(cfg):
    (B, IN_DIM, S, I_DIM, O_DIM, NCP, steps) = cfg
    TOT = S + I_DIM + O_DIM
    nc = bacc.Bacc("TRN2", target_bir_lowering=False, debug=False,
                   num_devices=N_CORES)
    xwp = nc.dram_tensor("xwp", (P, (IN_DIM // P) * (B + S)), F16, kind="ExternalInput").ap()
    bp = nc.dram_tensor("bp", (P, S // P), F32, kind="ExternalInput").ap()
    Wpre = nc.dram_tensor("Wpre", (P, (TOT // P) * S), F16, kind="ExternalInput").ap()
    Wpk = nc.dram_tensor("Wpk", (TOT // P, P, TOT), F16, kind="ExternalInput").ap()
    WoT = nc.dram_tensor("WoT", (O_DIM // P, P, NCP), F16, kind="ExternalInput").ap()
    out = nc.dram_tensor("out", (B // P, P, NCP), F32, kind="ExternalOutput").ap()
    with ExitStack() as ctx, tile.TileContext(nc) as tc:
        with ExitStack() as inner:
            _emit(inner, tc, (xwp, bp, Wpre, Wpk, WoT, out), cfg)
    nc.compile()
    return nc


def _get_nc(cfg):
    if cfg not in _cache:
        _cache[cfg] = _build(cfg)
    return _cache[cfg]


def kernel(x, W, Wp, bp, Wo, bo, time_steps):
    x = np.asarray(x, dtype=np.float32)
    W = np.asarray(W, dtype=np.float32)
    Wp = np.asarray(Wp, dtype=np.float32)
    bp = np.asarray(bp, dtype=np.float32)
    Wo = np.asarray(Wo, dtype=np.float32)
    bo = np.asarray(bo, dtype=np.float32)
    steps = int(time_steps)

    B_full, IN_DIM = x.shape
    TOT = W.shape[0]
    S = Wp.shape[0]
    NCLS, O_DIM = Wo.shape
    assert B_full % (N_CORES * P) == 0 and IN_DIM % P == 0
    assert S % P == 0 and O_DIM % P == 0 and TOT % P == 0
    B = B_full // N_CORES
    NCP = ((NCLS + P - 1) // P) * P
    cfg = (B, IN_DIM, S, TOT - S - O_DIM, O_DIM, NCP, steps)

    nc = _get_nc(cfg)

    # ---- host packing (replicated operands)
    W16 = W.astype(np.float16)
    # Wpk[mt, p, kt*P + mf] = W[kt*P + p, mt*P + mf]
    Wpk = np.ascontiguousarray(
        W16.reshape(TOT // P, P, TOT // P, P).transpose(2, 1, 0, 3)
    ).reshape(TOT // P, P, TOT)
    # step-1 prefix block: rows 0:S of every panel, panel-contiguous
    Wpre = np.ascontiguousarray(
        Wpk[:, :, :S].transpose(1, 0, 2)).reshape(P, (TOT // P) * S)
    WpT16 = Wp.T.astype(np.float16).reshape(IN_DIM // P, P, S)
    bpt = np.ascontiguousarray(bp.reshape(S // P, P).T)  # [p, s_tile]
    WoTp = np.zeros((O_DIM, NCP), dtype=np.float16)
    WoTp[:, :NCLS] = Wo.T.astype(np.float16)
    WoT = WoTp.reshape(O_DIM // P, P, NCP)

    xT = x.T.astype(np.float16)  # (IN_DIM, B_full)
    in_maps = []
    for c in range(N_CORES):
        xc = xT[:, c * B:(c + 1) * B].reshape(IN_DIM // P, P, B)
        xwp = np.concatenate([xc, WpT16], axis=2)  # (INT, P, B+S)
        # big-row layout: [P, INT*(B+S)] so the load is 2 full-BW DMAs
        xwp = np.ascontiguousarray(xwp.transpose(1, 0, 2)).reshape(P, -1)
        in_maps.append({"xwp": xwp, "bp": bpt, "Wpre": Wpre, "Wpk": Wpk,
                        "WoT": WoT})

    global LAST_RESULT
    res = run_bass_kernel_spmd(nc, in_maps, core_ids=list(range(N_CORES)),
                               **RUN_KWARGS)
    LAST_RESULT = res

    outs = []
    for c in range(N_CORES):
        oc = res.results[c]["out"].reshape(B, NCP)[:, :NCLS]
        outs.append(oc)
    return (np.concatenate(outs, axis=0) + bo[None, :]).astype(np.float32)


# revision 31
# speedup vs baseline: 1.0041x; 1.0025x over previous
"""Trainium2 Bass kernel for the BasicRNN problem.

Computation (see harness reference):
    E  = x @ Wp.T + bp                       # (B, S) sensory drive
    Z0 = 0                                   # (B, TOTAL)
    for t in range(time_steps):
        inj = [E if t % 5 == 0 else 0 | 0 | 0]
        Z  = relu(Z @ W + inj)
    out = Z[:, S+I:] @ Wo.T + bo             # (B, num_classes)

Strategy: data-parallel over batch across 8 NeuronCores (256 rows/core).
Each core keeps the state TRANSPOSED in SBUF (ZT: [TOTAL, B_shard] as
32 [128, 256] tiles) so every step is
    ZT_next[m] = relu(sum_k W[k, m-block].T-free @ ZT[k] (+ E^T[m]))
with lhsT = a [128k, 128m] block of W in its natural layout — no
transposes anywhere.  The kernel is jointly PE- and DMA-limited when W
(32 MiB fp16) is streamed every step, so the last 12 of the 32
column panels (12 MiB) stay RESIDENT in SBUF after step 2 — later
steps stream only 20 panels and the final step (O block only) streams
none.  The step-1 row-prefix of W is host-packed into one contiguous
block that pre-loads into the resident region, so step 1 runs at the
PE floor with the big stream pool free to prefetch step-2 panels.
Head rules learned from traces: full-128-partition DMAs only
(partition-sliced ones crawl), the tiny bias DMA must be issued before
the 8 MiB prefix block (ring semaphores serialize issue order), the
ACT table and HAM clock ramp are pre-warmed, and the output-layer
matmuls in the last step lag one m-tile so they never wait on a relu.
Every step then runs at the 109.2 ns/matmul hardware floor
(N/2.4 GHz + NX issue) with zero stalls.  Max rel err ~8e-4 vs the
fp32 reference.  fp8/int8 were evaluated and rejected: e4m3 W+state
gives 7.5e-2 final error (gate is 2e-2); DoubleRow also only pays at
free-dim >= 512 which this batch shard cannot reach.
"""

import numpy as np
from contextlib import ExitStack

from concourse import bacc, tile, mybir
from concourse.bass_utils import run_bass_kernel_spmd

P = 128
N_CORES = 8
F32 = mybir.dt.float32
F16 = mybir.dt.float16
AF = mybir.ActivationFunctionType

_cache: dict = {}

# extra kwargs for run_bass_kernel_spmd (test harness sets e.g. trace=True)
RUN_KWARGS: dict = {}
LAST_RESULT = None

N_RES = 12  # W panels kept resident in SBUF (of MT total)


def _emit(ctx: ExitStack, tc, aps, cfg):
    (B, IN_DIM, S, I_DIM, O_DIM, NCP, steps) = cfg
    TOT = S + I_DIM + O_DIM
    KT = TOT // P           # contraction tiles per step
    MT = TOT // P           # output-block tiles per step
    ST = S // P             # sensory tiles
    OT = O_DIM // P         # O-block tiles
    O0 = (S + I_DIM) // P   # first m-tile of the O block
    INT = IN_DIM // P
    nc = tc.nc
    (xwp, bp, Wpre, Wpk, WoT, out) = aps

    n_res = min(N_RES, MT)
    RES0 = MT - n_res       # panels m >= RES0 are resident

    io_pool = ctx.enter_context(tc.tile_pool(name="io", bufs=1))
    state_pool = ctx.enter_context(tc.tile_pool(name="state", bufs=1))
    e_pool = ctx.enter_context(tc.tile_pool(name="e", bufs=1))
    w_pool = ctx.enter_context(tc.tile_pool(name="w", bufs=5))
    res_pool = ctx.enter_context(tc.tile_pool(name="res", bufs=1))
    ps_pool = ctx.enter_context(tc.tile_pool(name="ps", bufs=4, space="PSUM"))

    def psum_tile(n):
        # all PSUM tiles share one tag (one bank each, `bufs` slots)
        return ps_pool.tile([P, 512], F32, name="ps", tag="ps")[:, :n]
    o_pool = ctx.enter_context(tc.tile_pool(name="o", bufs=2))

    # one contiguous region for the resident panels; its first columns
    # double as the landing zone for the host-packed step-1 prefix block
    res_big = res_pool.tile([P, n_res * TOT], F16, name="res")
    res_sb = [res_big[:, j * TOT:(j + 1) * TOT] for j in range(n_res)]
    pre_cols = MT * ST * P
    use_pre = steps >= 2 and n_res * TOT >= pre_cols

    # ---- PE warm-up: dummy matmuls with no DMA deps run during the
    # initial input DMAs and trip the HAM clock un-throttle (~3.4 us of
    # sustained PE activity) so the real matmuls start at 2.4 GHz.
    # DVE memset (gpsimd takes ~7 us to start and stalls the warm-up).
    # NOTE: a uniform 109->131 ns/MM spacing regression was once seen
    # after growing wu; later evidence showed 131 ns = 256/2.0GHz + NX,
    # i.e. the P0 power-state downclock (PE at 2.0 GHz under sustained
    # load), which also appears with unchanged code. If spacing jumps
    # uniformly, idle the device ~2 min and re-measure before blaming
    # the kernel.
    wu = io_pool.tile([P, P], F16, name="wu")
    nc.vector.memset(wu[:], 0.0)
    # warm the ACT table too: the first ACTIVATE otherwise pays a
    # 1.3 us ACT_TABLE_LOAD on the critical path of the t=0 relu
    nc.scalar.activation(wu[:, :1], wu[:, :1], AF.Relu)
    for _ in range(32):
        wu_ps = psum_tile(P)
        nc.tensor.matmul(wu_ps[:], lhsT=wu[:], rhs=wu[:], start=True, stop=True)

    # ---- load the small operands in as few DMAs as possible.  x/Wp are
    # host-packed into one [P, INT*(B+S)] buffer so the load is two DMAs
    # with 20 KB-contiguous partition rows (full HBM bandwidth); its SBUF
    # space is later recycled for the step-1 W prefixes and the Wo tiles.
    wot_sb = []
    BS = B + S
    # bp FIRST: DMA issues serialize on ~9 rotating ring semaphores, so
    # this tiny transfer must not queue behind the 8 MiB prefix block
    # (the t=0 relu needs it as its bias)
    bp_t = io_pool.tile([P, ST], F32, name="bp_t")
    nc.sync.dma_start(bp_t[:], bp)
    bp_sb = [bp_t[:, i:i + 1] for i in range(ST)]
    xwp_t = io_pool.tile([P, INT * BS], F16, name="xwp_t")
    xt_sb = [xwp_t[:, i * BS:i * BS + B] for i in range(INT)]
    wpt_sb = [xwp_t[:, i * BS + B:(i + 1) * BS] for i in range(INT)]
    # per-k-chunk DMAs: full-128-partition transfers are the fast DMA
    # path (partition-sliced ones crawl), and chunking lets the E
    # projection start on chunk 0 while the rest stream in.  (Splitting
    # chunk 0 further was tried and is a net loss: the extra ~585 ns
    # Sync-queue issue slots delay every later DMA.)
    for i in range(INT):
        nc.sync.dma_start(xwp_t[:, i * BS:(i + 1) * BS],
                          xwp[:, i * BS:(i + 1) * BS])
    # step-1 prefix block (rows 0:S of every panel, host-packed
    # m-contiguous): lands in the resident region before step 1 needs it,
    # in column chunks so panels complete in consumption order
    if use_pre:
        CPC = 2 * ST * P           # 2 panels' prefixes per column chunk
        for c0 in range(0, pre_cols, CPC):
            cw = min(CPC, pre_cols - c0)
            nc.sync.dma_start(res_big[:, c0:c0 + cw], Wpre[:, c0:c0 + cw])
    n_slots = (INT * BS) // (ST * P)
    sm_used = [0]

    def sm_tile(width):
        # recycle a slot of the x/Wp buffer (rotating; Tile's subslice
        # dependency tracking serializes against any overlapping readers)
        s = sm_used[0] % n_slots
        sm_used[0] += 1
        return xwp_t[:, s * (ST * P): s * (ST * P) + width]

    # ---- E^T = Wp @ x_shard^T + bp  (fp32, [S, B] as ST tiles)
    # k-outer: each input k-tile is consumed as soon as its DMA lands,
    # so the E matmuls overlap their own input loads
    e_ps = [ps_pool.tile([P, 512], F32, name="pse", tag="ps" if m < ST // 2
            else "ps2")[:, :B] for m in range(ST)]
    for ki in range(INT):
        for m in range(ST):
            nc.tensor.matmul(
                e_ps[m][:],
                lhsT=wpt_sb[ki][:, m * P:(m + 1) * P],
                rhs=xt_sb[ki][:],
                start=(ki == 0),
                stop=(ki == INT - 1),
            )
    # ---- state: two ping-pong buffers of KT [128, B] fp16 tiles
    zt = [
        [state_pool.tile([P, B], F16, name=f"z{b}_{k}") for k in range(KT)]
        for b in (0, 1)
    ]

    # t = 0: Z1 = relu([E | 0 | 0]) straight from PSUM with the bias fused
    # into the activation (ACT), while DVE independently materializes the
    # un-relu'd E for the t=5 injection — two short parallel chains
    # instead of one 16-op serial ACT chain.
    e_sb = []
    for m in range(ST):
        if steps >= 1:
            nc.scalar.activation(zt[1][m][:], e_ps[m][:], AF.Relu,
                                 bias=bp_sb[m][:])
        et = e_pool.tile([P, B], F32, name=f"et{m}")
        nc.vector.tensor_scalar_add(et[:], e_ps[m][:], bp_sb[m][:])
        e_sb.append(et)

    fin = steps % 2
    chunks = [(bt, c0) for bt in range(B // P) for c0 in range(0, NCP, 512)]
    ps2_tiles = {}
    ot_full = {}

    def load_wot():
        for i in range(OT):
            if NCP <= ST * P and n_slots >= 1:
                t4 = sm_tile(NCP)
            else:
                t4 = io_pool.tile([P, NCP], F16, name=f"wot{i}")
            nc.sync.dma_start(t4[:, :NCP], WoT[i])
            wot_sb.append(t4)

    def emit_final_group(j, last):
        # one k-slice (state tile O0+j) of the output-layer matmuls; called
        # inside the last step so these fold into the recurrence pipeline
        for (bt, c0) in chunks:
            cw = min(512, NCP - c0)
            if j == 0:
                ps2_tiles[(bt, c0)] = ps_pool.tile(
                    [P, 512], F32, name="ps2", tag="ps2")[:, :cw]
            nc.tensor.matmul(
                ps2_tiles[(bt, c0)][:],
                lhsT=zt[fin][O0 + j][:, bt * P:(bt + 1) * P],
                rhs=wot_sb[j][:, c0:c0 + cw],
                start=(j == 0),
                stop=last,
            )
            if last:
                # stage both 512-col chunks of a batch tile into one
                # [P, NCP] buffer so the store is a single DMA with
                # 4 KB rows (2 KB-row transfers are the slow packet
                # class and each extra dma_start costs an issue slot)
                if c0 == 0:
                    ot_full[bt] = o_pool.tile([P, NCP], F32, name="obuf")
                ot = ot_full[bt]
                if (bt + c0 // 512) % 2 == 0:
                    nc.vector.tensor_copy(ot[:, c0:c0 + cw],
                                          ps2_tiles[(bt, c0)][:])
                else:
                    nc.scalar.activation(ot[:, c0:c0 + cw],
                                         ps2_tiles[(bt, c0)][:], AF.Copy)
                if c0 + cw >= NCP:
                    nc.sync.dma_start(out[bt], ot[:, :NCP])

    live = ST  # number of non-zero k tiles in the current state
    for t in range(1, steps):
        cur, nxt = t % 2, (t + 1) % 2
        is_last = t == steps - 1
        m_lo, m_hi = (O0, MT) if is_last else (0, MT)
        k_n = live
        inject = (t % 5 == 0)
        if t == steps - 2 or (is_last and not wot_sb):
            # output-layer weights: issue the DMAs a step early so they
            # sit ready when the last step starts draining
            load_wot()
        for m in range(m_lo, m_hi):
            if m >= RES0 and t >= 2:
                # resident panel: loaded once at t=2, reused afterwards
                wp = res_sb[m - RES0]
                if t == 2:
                    nc.sync.dma_start(wp[:, : k_n * P], Wpk[m, :, : k_n * P])
            elif t == 1 and use_pre:
                # step-1 prefix: already pre-loaded in the resident
                # region by the head DMA block (overwritten at t=2)
                wp = res_big[:, m * (ST * P):(m + 1) * (ST * P)]
            else:
                wp = w_pool.tile([P, TOT], F16, name="wp")
                nc.sync.dma_start(wp[:, : k_n * P], Wpk[m, :, : k_n * P])
            ps = psum_tile(B)
            for k in range(k_n):
                nc.tensor.matmul(
                    ps[:],
                    lhsT=wp[:, k * P:(k + 1) * P],
                    rhs=zt[cur][k][:],
                    start=(k == 0),
                    stop=(k == k_n - 1),
                )
            if inject and m < ST:
                nc.vector.tensor_add(ps[:], ps[:], e_sb[m][:])
            if m % 2 == 0:
                nc.scalar.activation(zt[nxt][m][:], ps[:], AF.Relu)
            else:
                nc.vector.tensor_scalar_max(zt[nxt][m][:], ps[:], 0.0)
            if is_last and m > m_lo:
                # lag the output-layer slice one m-tile so it never waits
                # on the relu that produced its lhsT
                emit_final_group(m - 1 - O0, last=False)
        if is_last:
            emit_final_group(m_hi - 1 - O0, last=True)
        live = KT

    if steps < 2:
        # the O block was never written; zero it and run the output layer
        for j in range(OT):
            nc.vector.memset(zt[fin][O0 + j][:], 0.0)
        if not wot_sb:
            load_wot()
        for j in range(OT):
            emit_final_group(j, last=(j == OT - 1))


def _build(cfg):
    (B, IN_DIM, S, I_DIM, O_DIM, NCP, steps) = cfg
    TOT = S + I_DIM + O_DIM
    nc = bacc.Bacc("TRN2", target_bir_lowering=False, debug=False,
                   num_devices=N_CORES)
    xwp = nc.dram_tensor("xwp", (P, (IN_DIM // P) * (B + S)), F16, kind="ExternalInput").ap()
    bp = nc.dram_tensor("bp", (P, S // P), F32, kind="ExternalInput").ap()
    Wpre = nc.dram_tensor("Wpre", (P, (TOT // P) * S), F16, kind="ExternalInput").ap()
    Wpk = nc.dram_tensor("Wpk", (TOT // P, P, TOT), F16, kind="ExternalInput").ap()
    WoT = nc.dram_tensor("WoT", (O_DIM // P, P, NCP), F16, kind="ExternalInput").ap()
    out = nc.dram_tensor("out", (B // P, P, NCP), F32, kind="ExternalOutput").ap()
    with ExitStack() as ctx, tile.TileContext(nc) as tc:
        with ExitStack() as inner:
            _emit(inner, tc, (xwp, bp, Wpre, Wpk, WoT, out), cfg)
    nc.compile()
    return nc


def _get_nc(cfg):
    if cfg not in _cache:
        _cache[cfg] = _build(cfg)
    return _cache[cfg]


def kernel(x, W, Wp, bp, Wo, bo, time_steps):
    x = np.asarray(x, dtype=np.float32)
    W = np.asarray(W, dtype=np.float32)
    Wp = np.asarray(Wp, dtype=np.float32)
    bp = np.asarray(bp, dtype=np.float32)
    Wo = np.asarray(Wo, dtype=np.float32)
    bo = np.asarray(bo, dtype=np.float32)
    steps = int(time_steps)

    B_full, IN_DIM = x.shape
    TOT = W.shape[0]
    S = Wp.shape[0]
    NCLS, O_DIM = Wo.shape
    assert B_full % (N_CORES * P) == 0 and IN_DIM % P == 0
    assert S % P == 0 and O_DIM % P == 0 and TOT % P == 0
    B = B_full // N_CORES
    NCP = ((NCLS + P - 1) // P) * P
    cfg = (B, IN_DIM, S, TOT - S - O_DIM, O_DIM, NCP, steps)

    nc = _get_nc(cfg)

    # ---- host packing (replicated operands)
    W16 = W.astype(np.float16)
    # Wpk[mt, p, kt*P + mf] = W[kt*P + p, mt*P + mf]
    Wpk = np.ascontiguousarray(
        W16.reshape(TOT // P, P, TOT // P, P).transpose(2, 1, 0, 3)
    ).reshape(TOT // P, P, TOT)
    # step-1 prefix block: rows 0:S of every panel, panel-contiguous
    Wpre = np.ascontiguousarray(
        Wpk[:, :, :S].transpose(1, 0, 2)).reshape(P, (TOT // P) * S)
    WpT16 = Wp.T.astype(np.float16).reshape(IN_DIM // P, P, S)
    bpt = np.ascontiguousarray(bp.reshape(S // P, P).T)  # [p, s_tile]
    WoTp = np.zeros((O_DIM, NCP), dtype=np.float16)
    WoTp[:, :NCLS] = Wo.T.astype(np.float16)
    WoT = WoTp.reshape(O_DIM // P, P, NCP)

    xT = x.T.astype(np.float16)  # (IN_DIM, B_full)
    in_maps = []
    for c in range(N_CORES):
        xc = xT[:, c * B:(c + 1) * B].reshape(IN_DIM // P, P, B)
        xwp = np.concatenate([xc, WpT16], axis=2)  # (INT, P, B+S)
        # big-row layout: [P, INT*(B+S)] so the load is 2 full-BW DMAs
        xwp = np.ascontiguousarray(xwp.transpose(1, 0, 2)).reshape(P, -1)
        in_maps.append({"xwp": xwp, "bp": bpt, "Wpre": Wpre, "Wpk": Wpk,
                        "WoT": WoT})

    global LAST_RESULT
    res = run_bass_kernel_spmd(nc, in_maps, core_ids=list(range(N_CORES)),
                               **RUN_KWARGS)
    LAST_RESULT = res

    outs = []
    for c in range(N_CORES):
        oc = res.results[c]["out"].reshape(B, NCP)[:, :NCLS]
        outs.append(oc)
    return (np.concatenate(outs, axis=0) + bo[None, :]).astype(np.float32)


# revision 33
# speedup vs baseline: 1.0051x; 1.0010x over previous
"""Trainium2 Bass kernel for the BasicRNN problem.

Computation (see harness reference):
    E  = x @ Wp.T + bp                       # (B, S) sensory drive
    Z0 = 0                                   # (B, TOTAL)
    for t in range(time_steps):
        inj = [E if t % 5 == 0 else 0 | 0 | 0]
        Z  = relu(Z @ W + inj)
    out = Z[:, S+I:] @ Wo.T + bo             # (B, num_classes)

Strategy: data-parallel over batch across 8 NeuronCores (256 rows/core).
Each core keeps the state TRANSPOSED in SBUF (ZT: [TOTAL, B_shard] as
32 [128, 256] tiles) so every step is
    ZT_next[m] = relu(sum_k W[k, m-block].T-free @ ZT[k] (+ E^T[m]))
with lhsT = a [128k, 128m] block of W in its natural layout — no
transposes anywhere.  The kernel is jointly PE- and DMA-limited when W
(32 MiB fp16) is streamed every step, so the last 12 of the 32
column panels (12 MiB) stay RESIDENT in SBUF after step 2 — later
steps stream only 20 panels and the final step (O block only) streams
none.  The step-1 row-prefix of W is host-packed into one contiguous
block that pre-loads into the resident region, so step 1 runs at the
PE floor with the big stream pool free to prefetch step-2 panels.
Head rules learned from traces: full-128-partition DMAs only
(partition-sliced ones crawl), the tiny bias DMA must be issued before
the 8 MiB prefix block (ring semaphores serialize issue order), the
ACT table and HAM clock ramp are pre-warmed, and the output-layer
matmuls in the last step lag one m-tile so they never wait on a relu.
Every step then runs at the 109.2 ns/matmul hardware floor
(N/2.4 GHz + NX issue) with zero stalls.  Max rel err ~8e-4 vs the
fp32 reference.  fp8/int8 were evaluated and rejected: e4m3 W+state
gives 7.5e-2 final error (gate is 2e-2); DoubleRow also only pays at
free-dim >= 512 which this batch shard cannot reach.
"""

import numpy as np
from contextlib import ExitStack

from concourse import bacc, tile, mybir
from concourse.bass_utils import run_bass_kernel_spmd

P = 128
N_CORES = 8
F32 = mybir.dt.float32
F16 = mybir.dt.float16
AF = mybir.ActivationFunctionType

_cache: dict = {}

# extra kwargs for run_bass_kernel_spmd (test harness sets e.g. trace=True)
RUN_KWARGS: dict = {}
LAST_RESULT = None

N_RES = 12  # W panels kept resident in SBUF (of MT total)


def _emit(ctx: ExitStack, tc, aps, cfg):
    (B, IN_DIM, S, I_DIM, O_DIM, NCP, steps) = cfg
    TOT = S + I_DIM + O_DIM
    KT = TOT // P           # contraction tiles per step
    MT = TOT // P           # output-block tiles per step
    ST = S // P             # sensory tiles
    OT = O_DIM // P         # O-block tiles
    O0 = (S + I_DIM) // P   # first m-tile of the O block
    INT = IN_DIM // P
    nc = tc.nc
    (xwp, bp, Wpre, Wpk, WoT, out) = aps

    n_res = min(N_RES, MT)
    RES0 = MT - n_res       # panels m >= RES0 are resident

    io_pool = ctx.enter_context(tc.tile_pool(name="io", bufs=1))
    state_pool = ctx.enter_context(tc.tile_pool(name="state", bufs=1))
    e_pool = ctx.enter_context(tc.tile_pool(name="e", bufs=1))
    w_pool = ctx.enter_context(tc.tile_pool(name="w", bufs=5))
    res_pool = ctx.enter_context(tc.tile_pool(name="res", bufs=1))
    ps_pool = ctx.enter_context(tc.tile_pool(name="ps", bufs=4, space="PSUM"))

    def psum_tile(n):
        # all PSUM tiles share one tag (one bank each, `bufs` slots)
        return ps_pool.tile([P, 512], F32, name="ps", tag="ps")[:, :n]
    o_pool = ctx.enter_context(tc.tile_pool(name="o", bufs=2))

    # one contiguous region for the resident panels; its first columns
    # double as the landing zone for the host-packed step-1 prefix block
    res_big = res_pool.tile([P, n_res * TOT], F16, name="res")
    res_sb = [res_big[:, j * TOT:(j + 1) * TOT] for j in range(n_res)]
    pre_cols = MT * ST * P
    use_pre = steps >= 2 and n_res * TOT >= pre_cols

    # ---- PE warm-up: dummy matmuls with no DMA deps run during the
    # initial input DMAs and trip the HAM clock un-throttle (~3.4 us of
    # sustained PE activity) so the real matmuls start at 2.4 GHz.
    # DVE memset (gpsimd takes ~7 us to start and stalls the warm-up).
    # NOTE: a uniform 109->131 ns/MM spacing regression was once seen
    # after growing wu; later evidence showed 131 ns = 256/2.0GHz + NX,
    # i.e. the P0 power-state downclock (PE at 2.0 GHz under sustained
    # load), which also appears with unchanged code. If spacing jumps
    # uniformly, idle the device ~2 min and re-measure before blaming
    # the kernel.
    wu = io_pool.tile([P, P], F16, name="wu")
    nc.vector.memset(wu[:], 0.0)
    # warm the ACT table too: the first ACTIVATE otherwise pays a
    # 1.3 us ACT_TABLE_LOAD on the critical path of the t=0 relu
    nc.scalar.activation(wu[:, :1], wu[:, :1], AF.Relu)
    for _ in range(32):
        wu_ps = psum_tile(P)
        nc.tensor.matmul(wu_ps[:], lhsT=wu[:], rhs=wu[:], start=True, stop=True)

    # ---- load the small operands in as few DMAs as possible.  x/Wp are
    # host-packed into one [P, INT*(B+S)] buffer so the load is two DMAs
    # with 20 KB-contiguous partition rows (full HBM bandwidth); its SBUF
    # space is later recycled for the step-1 W prefixes and the Wo tiles.
    wot_sb = []
    BS = B + S
    # bp FIRST: DMA issues serialize on ~9 rotating ring semaphores, so
    # this tiny transfer must not queue behind the 8 MiB prefix block
    # (the t=0 relu needs it as its bias)
    bp_t = io_pool.tile([P, ST], F32, name="bp_t")
    nc.sync.dma_start(bp_t[:], bp)
    bp_sb = [bp_t[:, i:i + 1] for i in range(ST)]
    xwp_t = io_pool.tile([P, INT * BS], F16, name="xwp_t")
    xt_sb = [xwp_t[:, i * BS:i * BS + B] for i in range(INT)]
    wpt_sb = [xwp_t[:, i * BS + B:(i + 1) * BS] for i in range(INT)]
    # per-k-chunk DMAs: full-128-partition transfers are the fast DMA
    # path (partition-sliced ones crawl), and chunking lets the E
    # projection start on chunk 0 while the rest stream in.  (Splitting
    # chunk 0 further was tried and is a net loss: the extra ~585 ns
    # Sync-queue issue slots delay every later DMA.)
    for i in range(INT):
        nc.sync.dma_start(xwp_t[:, i * BS:(i + 1) * BS],
                          xwp[:, i * BS:(i + 1) * BS])
    # step-1 prefix block (rows 0:S of every panel, host-packed
    # m-contiguous): lands in the resident region before step 1 needs it,
    # in column chunks so panels complete in consumption order
    if use_pre:
        CPC = 2 * ST * P           # 2 panels' prefixes per column chunk
        for c0 in range(0, pre_cols, CPC):
            cw = min(CPC, pre_cols - c0)
            nc.sync.dma_start(res_big[:, c0:c0 + cw], Wpre[:, c0:c0 + cw])
    n_slots = (INT * BS) // (ST * P)
    sm_used = [0]

    def sm_tile(width):
        # recycle a slot of the x/Wp buffer (rotating; Tile's subslice
        # dependency tracking serializes against any overlapping readers)
        s = sm_used[0] % n_slots
        sm_used[0] += 1
        return xwp_t[:, s * (ST * P): s * (ST * P) + width]

    # ---- E^T = Wp @ x_shard^T + bp  (fp32, [S, B] as ST tiles)
    # k-outer: each input k-tile is consumed as soon as its DMA lands,
    # so the E matmuls overlap their own input loads
    e_ps = [ps_pool.tile([P, 512], F32, name="pse", tag="ps" if m < ST // 2
            else "ps2")[:, :B] for m in range(ST)]
    for ki in range(INT):
        for m in range(ST):
            nc.tensor.matmul(
                e_ps[m][:],
                lhsT=wpt_sb[ki][:, m * P:(m + 1) * P],
                rhs=xt_sb[ki][:],
                start=(ki == 0),
                stop=(ki == INT - 1),
            )
    # ---- state: two ping-pong buffers of KT [128, B] fp16 tiles
    zt = [
        [state_pool.tile([P, B], F16, name=f"z{b}_{k}") for k in range(KT)]
        for b in (0, 1)
    ]

    # t = 0: Z1 = relu([E | 0 | 0]) straight from PSUM with the bias fused
    # into the activation (ACT), while DVE independently materializes the
    # un-relu'd E for the t=5 injection — two short parallel chains
    # instead of one 16-op serial ACT chain.
    e_sb = []
    for m in range(ST):
        if steps >= 1:
            nc.scalar.activation(zt[1][m][:], e_ps[m][:], AF.Relu,
                                 bias=bp_sb[m][:])
        et = e_pool.tile([P, B], F32, name=f"et{m}")
        nc.vector.tensor_scalar_add(et[:], e_ps[m][:], bp_sb[m][:])
        e_sb.append(et)

    fin = steps % 2
    chunks = [(bt, c0) for bt in range(B // P) for c0 in range(0, NCP, 512)]
    ps2_tiles = {}
    ot_full = {}

    def load_wot():
        for i in range(OT):
            if NCP <= ST * P and n_slots >= 1:
                t4 = sm_tile(NCP)
            else:
                t4 = io_pool.tile([P, NCP], F16, name=f"wot{i}")
            nc.sync.dma_start(t4[:, :NCP], WoT[i])
            wot_sb.append(t4)

    def emit_final_group(j, last):
        # one k-slice (state tile O0+j) of the output-layer matmuls; called
        # inside the last step so these fold into the recurrence pipeline
        for (bt, c0) in chunks:
            cw = min(512, NCP - c0)
            if j == 0:
                ps2_tiles[(bt, c0)] = ps_pool.tile(
                    [P, 512], F32, name="ps2", tag="ps2")[:, :cw]
            nc.tensor.matmul(
                ps2_tiles[(bt, c0)][:],
                lhsT=zt[fin][O0 + j][:, bt * P:(bt + 1) * P],
                rhs=wot_sb[j][:, c0:c0 + cw],
                start=(j == 0),
                stop=last,
            )
            if last:
                # stage both 512-col chunks of a batch tile into one
                # [P, NCP] buffer so the store is a single DMA per batch
                # tile (each extra dma_start costs an issue slot); fp16
                # staging halves the drain bytes on the exec critical
                # path (~1e-4 extra error vs the 2e-2 gate)
                if c0 == 0:
                    ot_full[bt] = o_pool.tile([P, NCP], F16, name="obuf")
                ot = ot_full[bt]
                if (bt + c0 // 512) % 2 == 0:
                    nc.vector.tensor_copy(ot[:, c0:c0 + cw],
                                          ps2_tiles[(bt, c0)][:])
                else:
                    nc.scalar.activation(ot[:, c0:c0 + cw],
                                         ps2_tiles[(bt, c0)][:], AF.Copy)
                if c0 + cw >= NCP:
                    nc.sync.dma_start(out[bt], ot[:, :NCP])

    live = ST  # number of non-zero k tiles in the current state
    for t in range(1, steps):
        cur, nxt = t % 2, (t + 1) % 2
        is_last = t == steps - 1
        m_lo, m_hi = (O0, MT) if is_last else (0, MT)
        k_n = live
        inject = (t % 5 == 0)
        if t == steps - 2 or (is_last and not wot_sb):
            # output-layer weights: issue the DMAs a step early so they
            # sit ready when the last step starts draining
            load_wot()
        for m in range(m_lo, m_hi):
            if m >= RES0 and t >= 2:
                # resident panel: loaded once at t=2, reused afterwards
                wp = res_sb[m - RES0]
                if t == 2:
                    nc.sync.dma_start(wp[:, : k_n * P], Wpk[m, :, : k_n * P])
            elif t == 1 and use_pre:
                # step-1 prefix: already pre-loaded in the resident
                # region by the head DMA block (overwritten at t=2)
                wp = res_big[:, m * (ST * P):(m + 1) * (ST * P)]
            else:
                wp = w_pool.tile([P, TOT], F16, name="wp")
                nc.sync.dma_start(wp[:, : k_n * P], Wpk[m, :, : k_n * P])
            ps = psum_tile(B)
            for k in range(k_n):
                nc.tensor.matmul(
                    ps[:],
                    lhsT=wp[:, k * P:(k + 1) * P],
                    rhs=zt[cur][k][:],
                    start=(k == 0),
                    stop=(k == k_n - 1),
                )
            if inject and m < ST:
                nc.vector.tensor_add(ps[:], ps[:], e_sb[m][:])
            if m % 2 == 0:
                nc.scalar.activation(zt[nxt][m][:], ps[:], AF.Relu)
            else:
                nc.vector.tensor_scalar_max(zt[nxt][m][:], ps[:], 0.0)
            if is_last and m > m_lo:
                # lag the output-layer slice one m-tile so it never waits
                # on the relu that produced its lhsT
                emit_final_group(m - 1 - O0, last=False)
        if is_last:
            emit_final_group(m_hi - 1 - O0, last=True)
        live = KT

    if steps < 2:
        # the O block was never written; zero it and run the output layer
        for j in range(OT):
            nc.vector.memset(zt[fin][O0 + j][:], 0.0)
        if not wot_sb:
            load_wot()
        for j in range(OT):
            emit_final_group(j, last=(j == OT - 1))


def _build(cfg):
    (B, IN_DIM, S, I_DIM, O_DIM, NCP, steps) = cfg
    TOT = S + I_DIM + O_DIM
    nc = bacc.Bacc("TRN2", target_bir_lowering=False, debug=False,
                   num_devices=N_CORES)
    xwp = nc.dram_tensor("xwp", (P, (IN_DIM // P) * (B + S)), F16, kind="ExternalInput").ap()
    bp = nc.dram_tensor("bp", (P, S // P), F32, kind="ExternalInput").ap()
    Wpre = nc.dram_tensor("Wpre", (P, (TOT // P) * S), F16, kind="ExternalInput").ap()
    Wpk = nc.dram_tensor("Wpk", (TOT // P, P, TOT), F16, kind="ExternalInput").ap()
    WoT = nc.dram_tensor("WoT", (O_DIM // P, P, NCP), F16, kind="ExternalInput").ap()
    out = nc.dram_tensor("out", (B // P, P, NCP), F16, kind="ExternalOutput").ap()
    with ExitStack() as ctx, tile.TileContext(nc) as tc:
        with ExitStack() as inner:
            _emit(inner, tc, (xwp, bp, Wpre, Wpk, WoT, out), cfg)
    nc.compile()
    return nc


def _get_nc(cfg):
    if cfg not in _cache:
        _cache[cfg] = _build(cfg)
    return _cache[cfg]


def kernel(x, W, Wp, bp, Wo, bo, time_steps):
    x = np.asarray(x, dtype=np.float32)
    W = np.asarray(W, dtype=np.float32)
    Wp = np.asarray(Wp, dtype=np.float32)
    bp = np.asarray(bp, dtype=np.float32)
    Wo = np.asarray(Wo, dtype=np.float32)
    bo = np.asarray(bo, dtype=np.float32)
    steps = int(time_steps)

    B_full, IN_DIM = x.shape
    TOT = W.shape[0]
    S = Wp.shape[0]
    NCLS, O_DIM = Wo.shape
    assert B_full % (N_CORES * P) == 0 and IN_DIM % P == 0
    assert S % P == 0 and O_DIM % P == 0 and TOT % P == 0
    B = B_full // N_CORES
    NCP = ((NCLS + P - 1) // P) * P
    cfg = (B, IN_DIM, S, TOT - S - O_DIM, O_DIM, NCP, steps)

    nc = _get_nc(cfg)

    # ---- host packing (replicated operands)
    W16 = W.astype(np.float16)
    # Wpk[mt, p, kt*P + mf] = W[kt*P + p, mt*P + mf]
    Wpk = np.ascontiguousarray(
        W16.reshape(TOT // P, P, TOT // P, P).transpose(2, 1, 0, 3)
    ).reshape(TOT // P, P, TOT)
    # step-1 prefix block: rows 0:S of every panel, panel-contiguous
    Wpre = np.ascontiguousarray(
        Wpk[:, :, :S].transpose(1, 0, 2)).reshape(P, (TOT // P) * S)
    WpT16 = Wp.T.astype(np.float16).reshape(IN_DIM // P, P, S)
    bpt = np.ascontiguousarray(bp.reshape(S // P, P).T)  # [p, s_tile]
    WoTp = np.zeros((O_DIM, NCP), dtype=np.float16)
    WoTp[:, :NCLS] = Wo.T.astype(np.float16)
    WoT = WoTp.reshape(O_DIM // P, P, NCP)

    xT = x.T.astype(np.float16)  # (IN_DIM, B_full)
    in_maps = []
    for c in range(N_CORES):
        xc = xT[:, c * B:(c + 1) * B].reshape(IN_DIM // P, P, B)
        xwp = np.concatenate([xc, WpT16], axis=2)  # (INT, P, B+S)
        # big-row layout: [P, INT*(B+S)] so the load is 2 full-BW DMAs
        xwp = np.ascontiguousarray(xwp.transpose(1, 0, 2)).reshape(P, -1)
        in_maps.append({"xwp": xwp, "bp": bpt, "Wpre": Wpre, "Wpk": Wpk,
                        "WoT": WoT})

    global LAST_RESULT
    res = run_bass_kernel_spmd(nc, in_maps, core_ids=list(range(N_CORES)),
                               **RUN_KWARGS)
    LAST_RESULT = res

    outs = []
    for c in range(N_CORES):
        oc = res.results[c]["out"].reshape(B, NCP)[:, :NCLS]
        outs.append(oc)
    return (np.concatenate(outs, axis=0) + bo[None, :]).astype(np.float32)


# revision 34
# speedup vs baseline: 1.0054x; 1.0003x over previous
"""Trainium2 Bass kernel for the BasicRNN problem.

Computation (see harness reference):
    E  = x @ Wp.T + bp                       # (B, S) sensory drive
    Z0 = 0                                   # (B, TOTAL)
    for t in range(time_steps):
        inj = [E if t % 5 == 0 else 0 | 0 | 0]
        Z  = relu(Z @ W + inj)
    out = Z[:, S+I:] @ Wo.T + bo             # (B, num_classes)

Strategy: data-parallel over batch across 8 NeuronCores (256 rows/core).
Each core keeps the state TRANSPOSED in SBUF (ZT: [TOTAL, B_shard] as
32 [128, 256] tiles) so every step is
    ZT_next[m] = relu(sum_k W[k, m-block].T-free @ ZT[k] (+ E^T[m]))
with lhsT = a [128k, 128m] block of W in its natural layout — no
transposes anywhere.  The kernel is jointly PE- and DMA-limited when W
(32 MiB fp16) is streamed every step, so the last 12 of the 32
column panels (12 MiB) stay RESIDENT in SBUF after step 2 — later
steps stream only 20 panels and the final step (O block only) streams
none.  The step-1 row-prefix of W is host-packed into one contiguous
block that pre-loads into the resident region, so step 1 runs at the
PE floor with the big stream pool free to prefetch step-2 panels.
Head rules learned from traces: full-128-partition DMAs only
(partition-sliced ones crawl), the tiny bias DMA must be issued before
the 8 MiB prefix block (ring semaphores serialize issue order), the
ACT table and HAM clock ramp are pre-warmed, and the output-layer
matmuls in the last step lag one m-tile so they never wait on a relu.
Every step then runs at the 109.2 ns/matmul hardware floor
(N/2.4 GHz + NX issue) with zero stalls.  Max rel err ~8e-4 vs the
fp32 reference.  fp8/int8 were evaluated and rejected: e4m3 W+state
gives 7.5e-2 final error (gate is 2e-2); DoubleRow also only pays at
free-dim >= 512 which this batch shard cannot reach.
"""

import numpy as np
from contextlib import ExitStack

from concourse import bacc, tile, mybir
from concourse.bass_utils import run_bass_kernel_spmd

P = 128
N_CORES = 8
F32 = mybir.dt.float32
F16 = mybir.dt.float16
AF = mybir.ActivationFunctionType

_cache: dict = {}

# extra kwargs for run_bass_kernel_spmd (test harness sets e.g. trace=True)
RUN_KWARGS: dict = {}
LAST_RESULT = None

N_RES = 12  # W panels kept resident in SBUF (of MT total)


def _emit(ctx: ExitStack, tc, aps, cfg):
    (B, IN_DIM, S, I_DIM, O_DIM, NCP, steps) = cfg
    TOT = S + I_DIM + O_DIM
    KT = TOT // P           # contraction tiles per step
    MT = TOT // P           # output-block tiles per step
    ST = S // P             # sensory tiles
    OT = O_DIM // P         # O-block tiles
    O0 = (S + I_DIM) // P   # first m-tile of the O block
    INT = IN_DIM // P
    nc = tc.nc
    (xwp, bp, Wpre, Wpk, WoT, out) = aps

    n_res = min(N_RES, MT)
    RES0 = MT - n_res       # panels m >= RES0 are resident

    io_pool = ctx.enter_context(tc.tile_pool(name="io", bufs=1))
    state_pool = ctx.enter_context(tc.tile_pool(name="state", bufs=1))
    e_pool = ctx.enter_context(tc.tile_pool(name="e", bufs=1))
    w_pool = ctx.enter_context(tc.tile_pool(name="w", bufs=5))
    res_pool = ctx.enter_context(tc.tile_pool(name="res", bufs=1))
    ps_pool = ctx.enter_context(tc.tile_pool(name="ps", bufs=4, space="PSUM"))

    def psum_tile(n):
        # all PSUM tiles share one tag (one bank each, `bufs` slots)
        return ps_pool.tile([P, 512], F32, name="ps", tag="ps")[:, :n]
    o_pool = ctx.enter_context(tc.tile_pool(name="o", bufs=2))

    # one contiguous region for the resident panels; its first columns
    # double as the landing zone for the host-packed step-1 prefix block
    res_big = res_pool.tile([P, n_res * TOT], F16, name="res")
    res_sb = [res_big[:, j * TOT:(j + 1) * TOT] for j in range(n_res)]
    pre_cols = MT * ST * P
    use_pre = steps >= 2 and n_res * TOT >= pre_cols

    # ---- PE warm-up: dummy matmuls with no DMA deps run during the
    # initial input DMAs and trip the HAM clock un-throttle (~3.4 us of
    # sustained PE activity) so the real matmuls start at 2.4 GHz.
    # DVE memset (gpsimd takes ~7 us to start and stalls the warm-up).
    # NOTE: a uniform 109->131 ns/MM spacing regression was once seen
    # after growing wu; later evidence showed 131 ns = 256/2.0GHz + NX,
    # i.e. the P0 power-state downclock (PE at 2.0 GHz under sustained
    # load), which also appears with unchanged code. If spacing jumps
    # uniformly, idle the device ~2 min and re-measure before blaming
    # the kernel.
    wu = io_pool.tile([P, P], F16, name="wu")
    nc.vector.memset(wu[:], 0.0)
    # warm the ACT table too: the first ACTIVATE otherwise pays a
    # 1.3 us ACT_TABLE_LOAD on the critical path of the t=0 relu
    nc.scalar.activation(wu[:, :1], wu[:, :1], AF.Relu)
    for _ in range(32):
        wu_ps = psum_tile(P)
        nc.tensor.matmul(wu_ps[:], lhsT=wu[:], rhs=wu[:], start=True, stop=True)

    # ---- load the small operands in as few DMAs as possible.  x/Wp are
    # host-packed into one [P, INT*(B+S)] buffer so the load is two DMAs
    # with 20 KB-contiguous partition rows (full HBM bandwidth); its SBUF
    # space is later recycled for the step-1 W prefixes and the Wo tiles.
    wot_sb = []
    BS = B + S
    # bp FIRST: DMA issues serialize on ~9 rotating ring semaphores, so
    # this tiny transfer must not queue behind the 8 MiB prefix block
    # (the t=0 relu needs it as its bias)
    bp_t = io_pool.tile([P, ST], F32, name="bp_t")
    nc.sync.dma_start(bp_t[:], bp)
    bp_sb = [bp_t[:, i:i + 1] for i in range(ST)]
    xwp_t = io_pool.tile([P, INT * BS], F16, name="xwp_t")
    xt_sb = [xwp_t[:, i * BS:i * BS + B] for i in range(INT)]
    wpt_sb = [xwp_t[:, i * BS + B:(i + 1) * BS] for i in range(INT)]
    # per-k-chunk DMAs: full-128-partition transfers are the fast DMA
    # path (partition-sliced ones crawl), and chunking lets the E
    # projection start on chunk 0 while the rest stream in.  (Splitting
    # chunk 0 further was tried and is a net loss: the extra ~585 ns
    # Sync-queue issue slots delay every later DMA.)
    for i in range(INT):
        nc.sync.dma_start(xwp_t[:, i * BS:(i + 1) * BS],
                          xwp[:, i * BS:(i + 1) * BS])
    # step-1 prefix block (rows 0:S of every panel, host-packed
    # m-contiguous): lands in the resident region before step 1 needs it,
    # in column chunks so panels complete in consumption order
    if use_pre:
        CPC = 2 * ST * P           # 2 panels' prefixes per column chunk
        for c0 in range(0, pre_cols, CPC):
            cw = min(CPC, pre_cols - c0)
            nc.sync.dma_start(res_big[:, c0:c0 + cw], Wpre[:, c0:c0 + cw])
    n_slots = (INT * BS) // (ST * P)
    sm_used = [0]

    def sm_tile(width):
        # recycle a slot of the x/Wp buffer (rotating; Tile's subslice
        # dependency tracking serializes against any overlapping readers)
        s = sm_used[0] % n_slots
        sm_used[0] += 1
        return xwp_t[:, s * (ST * P): s * (ST * P) + width]

    # ---- E^T = Wp @ x_shard^T + bp  (fp32, [S, B] as ST tiles)
    # k-outer: each input k-tile is consumed as soon as its DMA lands,
    # so the E matmuls overlap their own input loads
    e_ps = [ps_pool.tile([P, 512], F32, name="pse", tag="ps" if m < ST // 2
            else "ps2")[:, :B] for m in range(ST)]
    for ki in range(INT):
        for m in range(ST):
            nc.tensor.matmul(
                e_ps[m][:],
                lhsT=wpt_sb[ki][:, m * P:(m + 1) * P],
                rhs=xt_sb[ki][:],
                start=(ki == 0),
                stop=(ki == INT - 1),
            )
    # ---- state: two ping-pong buffers of KT [128, B] fp16 tiles
    zt = [
        [state_pool.tile([P, B], F16, name=f"z{b}_{k}") for k in range(KT)]
        for b in (0, 1)
    ]

    # t = 0: Z1 = relu([E | 0 | 0]) straight from PSUM with the bias fused
    # into the activation (ACT), while DVE independently materializes the
    # un-relu'd E for the t=5 injection — two short parallel chains
    # instead of one 16-op serial ACT chain.
    e_sb = []
    for m in range(ST):
        if steps >= 1:
            nc.scalar.activation(zt[1][m][:], e_ps[m][:], AF.Relu,
                                 bias=bp_sb[m][:])
        et = e_pool.tile([P, B], F32, name=f"et{m}")
        nc.vector.tensor_scalar_add(et[:], e_ps[m][:], bp_sb[m][:])
        e_sb.append(et)

    fin = steps % 2
    chunks = [(bt, c0) for bt in range(B // P) for c0 in range(0, NCP, 512)]
    ps2_tiles = {}
    ot_full = {}

    def load_wot():
        for i in range(OT):
            if NCP <= ST * P and n_slots >= 1:
                t4 = sm_tile(NCP)
            else:
                t4 = io_pool.tile([P, NCP], F16, name=f"wot{i}")
            nc.sync.dma_start(t4[:, :NCP], WoT[i])
            wot_sb.append(t4)

    def emit_final_group(j, last):
        # one k-slice (state tile O0+j) of the output-layer matmuls; called
        # inside the last step so these fold into the recurrence pipeline
        for (bt, c0) in chunks:
            cw = min(512, NCP - c0)
            if j == 0:
                ps2_tiles[(bt, c0)] = ps_pool.tile(
                    [P, 512], F32, name="ps2", tag="ps2")[:, :cw]
            nc.tensor.matmul(
                ps2_tiles[(bt, c0)][:],
                lhsT=zt[fin][O0 + j][:, bt * P:(bt + 1) * P],
                rhs=wot_sb[j][:, c0:c0 + cw],
                start=(j == 0),
                stop=last,
            )
            if last:
                # stage both 512-col chunks of a batch tile into one
                # [P, NCP] buffer so the store is a single DMA per batch
                # tile (each extra dma_start costs an issue slot); fp16
                # staging halves the drain bytes on the exec critical
                # path (~1e-4 extra error vs the 2e-2 gate)
                if c0 == 0:
                    ot_full[bt] = o_pool.tile([P, NCP], F16, name="obuf")
                ot = ot_full[bt]
                if (bt + c0 // 512) % 2 == 0:
                    nc.vector.tensor_copy(ot[:, c0:c0 + cw],
                                          ps2_tiles[(bt, c0)][:])
                else:
                    nc.scalar.activation(ot[:, c0:c0 + cw],
                                         ps2_tiles[(bt, c0)][:], AF.Copy)
                if c0 + cw >= NCP:
                    nc.sync.dma_start(out[bt], ot[:, :NCP])

    live = ST  # number of non-zero k tiles in the current state
    for t in range(1, steps):
        cur, nxt = t % 2, (t + 1) % 2
        is_last = t == steps - 1
        m_lo, m_hi = (O0, MT) if is_last else (0, MT)
        k_n = live
        inject = (t % 5 == 0)
        if t == steps - 2 or (is_last and not wot_sb):
            # output-layer weights: issue the DMAs a step early so they
            # sit ready when the last step starts draining
            load_wot()
        for m in range(m_lo, m_hi):
            if m >= RES0 and t >= 2:
                # resident panel: loaded once at t=2, reused afterwards
                wp = res_sb[m - RES0]
                if t == 2:
                    nc.sync.dma_start(wp[:, : k_n * P], Wpk[m, :, : k_n * P])
            elif t == 1 and use_pre:
                # step-1 prefix: already pre-loaded in the resident
                # region by the head DMA block (overwritten at t=2)
                wp = res_big[:, m * (ST * P):(m + 1) * (ST * P)]
            else:
                wp = w_pool.tile([P, TOT], F16, name="wp")
                nc.sync.dma_start(wp[:, : k_n * P], Wpk[m, :, : k_n * P])
            ps = psum_tile(B)
            for k in range(k_n):
                nc.tensor.matmul(
                    ps[:],
                    lhsT=wp[:, k * P:(k + 1) * P],
                    rhs=zt[cur][k][:],
                    start=(k == 0),
                    stop=(k == k_n - 1),
                )
            if inject and m < ST:
                nc.vector.tensor_add(ps[:], ps[:], e_sb[m][:])
            if is_last and m == m_hi - 1 and B == 2 * P:
                # the very last relu gates the final output-layer MMs,
                # which read it in [P]-column halves per batch tile:
                # split it across both engines so they start sooner
                nc.scalar.activation(zt[nxt][m][:, :P], ps[:, :P], AF.Relu)
                nc.vector.tensor_scalar_max(zt[nxt][m][:, P:], ps[:, P:], 0.0)
            elif m % 2 == 0:
                nc.scalar.activation(zt[nxt][m][:], ps[:], AF.Relu)
            else:
                nc.vector.tensor_scalar_max(zt[nxt][m][:], ps[:], 0.0)
            if is_last and m > m_lo:
                # lag the output-layer slice one m-tile so it never waits
                # on the relu that produced its lhsT
                emit_final_group(m - 1 - O0, last=False)
        if is_last:
            emit_final_group(m_hi - 1 - O0, last=True)
        live = KT

    if steps < 2:
        # the O block was never written; zero it and run the output layer
        for j in range(OT):
            nc.vector.memset(zt[fin][O0 + j][:], 0.0)
        if not wot_sb:
            load_wot()
        for j in range(OT):
            emit_final_group(j, last=(j == OT - 1))


def _build(cfg):
    (B, IN_DIM, S, I_DIM, O_DIM, NCP, steps) = cfg
    TOT = S + I_DIM + O_DIM
    nc = bacc.Bacc("TRN2", target_bir_lowering=False, debug=False,
                   num_devices=N_CORES)
    xwp = nc.dram_tensor("xwp", (P, (IN_DIM // P) * (B + S)), F16, kind="ExternalInput").ap()
    bp = nc.dram_tensor("bp", (P, S // P), F32, kind="ExternalInput").ap()
    Wpre = nc.dram_tensor("Wpre", (P, (TOT // P) * S), F16, kind="ExternalInput").ap()
    Wpk = nc.dram_tensor("Wpk", (TOT // P, P, TOT), F16, kind="ExternalInput").ap()
    WoT = nc.dram_tensor("WoT", (O_DIM // P, P, NCP), F16, kind="ExternalInput").ap()
    out = nc.dram_tensor("out", (B // P, P, NCP), F16, kind="ExternalOutput").ap()
    with ExitStack() as ctx, tile.TileContext(nc) as tc:
        with ExitStack() as inner:
            _emit(inner, tc, (xwp, bp, Wpre, Wpk, WoT, out), cfg)
    nc.compile()
    return nc


def _get_nc(cfg):
    if cfg not in _cache:
        _cache[cfg] = _build(cfg)
    return _cache[cfg]


def kernel(x, W, Wp, bp, Wo, bo, time_steps):
    x = np.asarray(x, dtype=np.float32)
    W = np.asarray(W, dtype=np.float32)
    Wp = np.asarray(Wp, dtype=np.float32)
    bp = np.asarray(bp, dtype=np.float32)
    Wo = np.asarray(Wo, dtype=np.float32)
    bo = np.asarray(bo, dtype=np.float32)
    steps = int(time_steps)

    B_full, IN_DIM = x.shape
    TOT = W.shape[0]
    S = Wp.shape[0]
    NCLS, O_DIM = Wo.shape
    assert B_full % (N_CORES * P) == 0 and IN_DIM % P == 0
    assert S % P == 0 and O_DIM % P == 0 and TOT % P == 0
    B = B_full // N_CORES
    NCP = ((NCLS + P - 1) // P) * P
    cfg = (B, IN_DIM, S, TOT - S - O_DIM, O_DIM, NCP, steps)

    nc = _get_nc(cfg)

    # ---- host packing (replicated operands)
    W16 = W.astype(np.float16)
    # Wpk[mt, p, kt*P + mf] = W[kt*P + p, mt*P + mf]
    Wpk = np.ascontiguousarray(
        W16.reshape(TOT // P, P, TOT // P, P).transpose(2, 1, 0, 3)
    ).reshape(TOT // P, P, TOT)
    # step-1 prefix block: rows 0:S of every panel, panel-contiguous
    Wpre = np.ascontiguousarray(
        Wpk[:, :, :S].transpose(1, 0, 2)).reshape(P, (TOT // P) * S)
    WpT16 = Wp.T.astype(np.float16).reshape(IN_DIM // P, P, S)
    bpt = np.ascontiguousarray(bp.reshape(S // P, P).T)  # [p, s_tile]
    WoTp = np.zeros((O_DIM, NCP), dtype=np.float16)
    WoTp[:, :NCLS] = Wo.T.astype(np.float16)
    WoT = WoTp.reshape(O_DIM // P, P, NCP)

    xT = x.T.astype(np.float16)  # (IN_DIM, B_full)
    in_maps = []
    for c in range(N_CORES):
        xc = xT[:, c * B:(c + 1) * B].reshape(IN_DIM // P, P, B)
        xwp = np.concatenate([xc, WpT16], axis=2)  # (INT, P, B+S)
        # big-row layout: [P, INT*(B+S)] so the load is 2 full-BW DMAs
        xwp = np.ascontiguousarray(xwp.transpose(1, 0, 2)).reshape(P, -1)
        in_maps.append({"xwp": xwp, "bp": bpt, "Wpre": Wpre, "Wpk": Wpk,
                        "WoT": WoT})

    global LAST_RESULT
    res = run_bass_kernel_spmd(nc, in_maps, core_ids=list(range(N_CORES)),
                               **RUN_KWARGS)
    LAST_RESULT = res

    outs = []
    for c in range(N_CORES):
        oc = res.results[c]["out"].reshape(B, NCP)[:, :NCLS]
        outs.append(oc)
    return (np.concatenate(outs, axis=0) + bo[None, :]).astype(np.float32)
